# revision 29
# baseline (speedup 1.0000x reference)
"""Canny edge filter (nms_detection) Trainium2 Bass kernel.

Full inputs: x [128, 512, 512] f32 (plus 1x1 gaussian + sobel kernels, which
are compile-time constants here). Output: [128, 512, 512] f32 binary edges.

Strategy: shard the 128 slices across 8 cores (16 per core). Each slice is
independent (3x3 stencils + per-slice max). All math is done in the
squared-magnitude domain (no sqrt / arctan2 needed):
  - gx, gy via fp32 TensorE matmuls with banded stencil matrices
    (vertical part) and column-shifted access patterns (horizontal part).
  - sqx, sqy via ScalarE Square (exact), msq = sqx + sqy.
  - NMS direction via comparisons: t^2*sqx <= sqy etc. (t = tan 22.5deg).
  - neighbor access via DMA partition-shifted copies of msq + col offsets.
  - per-slice max of msq == per-slice max of NMS'd mag^2 (the argmax always
    survives NMS), so thresholds are computed in pass A.
  - hysteresis: 3x3 box-sum of strong on PE in bf16 (exact for 0/1 data).
  - the short tail strips of 4 slices are packed into one 128-partition tile
    (32-partition groups, block-diagonal stencil matrices).
"""
import sys
import math
from contextlib import ExitStack
from types import SimpleNamespace

sys.path.insert(0, "/opt/trn_rl_repo")

import numpy as np
import ml_dtypes

import concourse.bass as bass
import concourse.bacc as bacc
import concourse.bass_isa as bass_isa
import concourse.mybir as mybir
import concourse.tile as tile
from concourse import bass_utils

F32 = mybir.dt.float32
BF16 = mybir.dt.bfloat16
U8 = mybir.dt.uint8
ALU = mybir.AluOpType
ACTF = mybir.ActivationFunctionType

D, H, W = 128, 512, 512
N_CORES = 8
D_SH = D // N_CORES

# constants matching the reference's f32 arithmetic boundaries
T2 = np.float32((math.sqrt(2.0) - 1.0) ** 2)          # tan^2(22.5 deg)
CSQ = np.float32(np.float64(np.float32(0.05)) ** 2)    # HIGH_T^2
DSQ = np.float32(np.float64(np.float32(0.01)) ** 2)    # LOW_T^2

EDGE_ROWS = 122  # edge rows produced per full strip (128 partitions - 6 halo)
GSTRIDE = 32     # partition stride of packed tail groups


def _band_mats(specs, ncols):
    """Build banded lhsT matrices [128, ncols] from (col, row, weight) spec fn.

    specs: list of (qstart, qcount) bands; each band places, for local q in
    [0, qcount): weights at rows qstart+q+dj for dj, w in the stencil."""
    vs = np.zeros((128, ncols), np.float32)
    vd = np.zeros((128, ncols), np.float32)
    t3 = np.zeros((128, ncols), np.float32)
    for (qs, qc) in specs:
        for j in range(qc):
            q = qs + j
            vs[q, q] = 1.0
            vs[q + 1, q] = 2.0
            vs[q + 2, q] = 1.0
            vd[q, q] = -1.0
            vd[q + 2, q] = 1.0
            # centered ones band, clipped to this group's rows
            if j > 0:
                t3[q - 1, q] = 1.0
            t3[q, q] = 1.0
            if j < qc - 1 or True:
                t3[q + 1, q] = 1.0
    return vs, -vs, vd, 2.0 * vd, t3.astype(ml_dtypes.bfloat16)


def _stencil_mats(h=H):
    """Regular + packed-tail matrices for height h."""
    strips = _strips(h)
    reg = _band_mats([(0, 126)], 126)
    mats = {"vs": reg[0], "vsn": reg[1], "vd": reg[2], "vd2": reg[3], "t3": reg[4]}
    if _pack_ok(h, 4):
        mt = strips[-1][1]
        pk = _band_mats([(g * GSTRIDE, mt) for g in range(4)], 128)
        mats.update({"vsp": pk[0], "vsnp": pk[1], "vdp": pk[2],
                     "vd2p": pk[3], "t3p": pk[4]})
    return mats


def _strips(h):
    """Strip table: (R0, M). R0 = grid row of x partition 0.
    msq rows R0+1 .. R0+M, edge rows R0+3 .. min(h, R0+2+EDGE_ROWS)."""
    n = max(1, math.ceil(h / EDGE_ROWS))
    out = []
    for s in range(n):
        r0 = EDGE_ROWS * s - 2
        m = min(126, (h + 2) - (EDGE_ROWS * s - 1) + 1)
        out.append((r0, m))
    return out


def _pack_ok(h, gsz):
    strips = _strips(h)
    return len(strips) >= 2 and strips[-1][1] + 2 <= GSTRIDE - 1


def build_nc(dsh, h, w, reps=1):
    """Build the per-core Bass program: x [dsh, h, w] -> out [dsh, h, w]."""
    gw = w + 2          # grid width
    xw = w + 6          # x tile width; x col t <-> grid col t-2
    mw = w + 4          # msq tile width; msq col t <-> grid col t-1
    blocks = []
    c = 1
    while c < gw + 1:
        bw = min(258, gw + 1 - c)
        blocks.append((c, bw))
        c += bw
    strips = _strips(h)
    pack = _pack_ok(h, 4) and dsh % 4 == 0
    gsz = 4 if pack else 1
    full_strips = strips[:-1] if pack else strips
    mats_np = _stencil_mats(h)

    nc = bacc.Bacc(trn_type="TRN2")
    x_d = nc.dram_tensor("x", [dsh, h, w], F32, kind="ExternalInput")
    mat_d = {}
    for name, arr in mats_np.items():
        dt = BF16 if arr.dtype == ml_dtypes.bfloat16 else F32
        mat_d[name] = nc.dram_tensor(name, list(arr.shape), dt, kind="ExternalInput")
    o_d = nc.dram_tensor("o", [dsh, h, w], F32, kind="ExternalOutput")

    with ExitStack() as ctx:
        tc = ctx.enter_context(tile.TileContext(nc))
        consts = ctx.enter_context(tc.tile_pool(name="consts", bufs=1))
        xp = ctx.enter_context(tc.tile_pool(name="xp", bufs=3))
        ps = ctx.enter_context(tc.tile_pool(name="ps", bufs=2, space="PSUM"))
        sqp = ctx.enter_context(tc.tile_pool(name="sqp", bufs=2))
        msqp = ctx.enter_context(tc.tile_pool(name="msqp", bufs=20))
        maskp = ctx.enter_context(tc.tile_pool(name="maskp", bufs=20))
        accp = ctx.enter_context(tc.tile_pool(name="accp", bufs=24))
        thrp = ctx.enter_context(tc.tile_pool(name="thrp", bufs=6))
        udp = ctx.enter_context(tc.tile_pool(name="udp", bufs=2))
        nmaxp = ctx.enter_context(tc.tile_pool(name="nmaxp", bufs=2))
        selp = ctx.enter_context(tc.tile_pool(name="selp", bufs=2))
        bfp = ctx.enter_context(tc.tile_pool(name="bfp", bufs=2))
        outp = ctx.enter_context(tc.tile_pool(name="outp", bufs=3))

        mat_s = {}
        for name, dram in mat_d.items():
            t = consts.tile(list(mats_np[name].shape),
                            BF16 if name.startswith("t3") else F32,
                            tag=name, name=f"c_{name}")
            nc.sync.dma_start(t, dram[:])
            mat_s[name] = t

        loop_ctx = tc.For_i(0, reps, 1) if reps > 1 else None
        if loop_ctx is not None:
            ctx.enter_context(loop_ctx)

        def emit_pass_a(job):
            """Pass A for one tile-job. Returns tiles namespace."""
            m = job.m
            xt = xp.tile([128, xw], F32, tag="x", name=f"x_{job.name}")
            # full clear unless a single segment's image rows + top-pad memset
            # cover all 128 partitions
            clear_x = not (len(job.segs) == 1 and job.segs[0][0] == 0
                           and min(h, job.segs[0][2] + 127) - job.segs[0][2] == 127)
            if clear_x:
                nc.gpsimd.memset(xt[:, :], 0.0)
            else:
                nc.gpsimd.memset(xt[:, 0:3], 0.0)
                nc.gpsimd.memset(xt[:, xw - 3:xw], 0.0)
                for (pstart, sl, r0, nx) in job.segs:
                    img_lo = max(1, r0) - r0
                    if img_lo > 0:
                        assert pstart == 0
                        nc.gpsimd.memset(xt[0:img_lo, :], 0.0)
            for (pstart, sl, r0, nx) in job.segs:
                img_lo = max(1, r0) - r0
                img_hi = min(h, r0 + nx - 1) - r0
                nc.sync.dma_start(
                    xt[pstart + img_lo:pstart + img_hi + 1, 3:w + 3],
                    x_d[sl, r0 + img_lo - 1:r0 + img_hi, 0:w])

            msq = msqp.tile([128, mw], F32, tag="msq", name=f"msq_{job.name}")
            sqx = sqp.tile([128, mw], F32, tag="sqx", name=f"sqx_{job.name}")
            sqy = sqp.tile([128, mw], F32, tag="sqy", name=f"sqy_{job.name}")
            sgx = sqp.tile([128, mw], BF16, tag="sgx", name=f"sgx_{job.name}")
            sgy = sqp.tile([128, mw], BF16, tag="sgy", name=f"sgy_{job.name}")
            vsn_m, vs_m, vd_m, vd2_m = job.mats
            for (c0, bw) in blocks:
                xl, xc, xr = c0, c0 + 1, c0 + 2
                gx = ps.tile([128, 258], F32, tag="gx", name=f"gx_{job.name}_{c0}")
                gy = ps.tile([128, 258], F32, tag="gy", name=f"gy_{job.name}_{c0}")
                nc.tensor.matmul(gx[0:m, 0:bw], vsn_m[:, 0:m], xt[:, xl:xl + bw],
                                 start=True, stop=False)
                nc.tensor.matmul(gx[0:m, 0:bw], vs_m[:, 0:m], xt[:, xr:xr + bw],
                                 start=False, stop=True)
                nc.tensor.matmul(gy[0:m, 0:bw], vd_m[:, 0:m], xt[:, xl:xl + bw],
                                 start=True, stop=False)
                nc.tensor.matmul(gy[0:m, 0:bw], vd2_m[:, 0:m], xt[:, xc:xc + bw],
                                 start=False, stop=False)
                nc.tensor.matmul(gy[0:m, 0:bw], vd_m[:, 0:m], xt[:, xr:xr + bw],
                                 start=False, stop=True)
                nc.scalar.activation(out=sqx[0:m, c0:c0 + bw], in_=gx[0:m, 0:bw],
                                     func=ACTF.Square)
                nc.scalar.activation(out=sqy[0:m, c0:c0 + bw], in_=gy[0:m, 0:bw],
                                     func=ACTF.Square)
                nc.scalar.activation(out=sgx[0:m, c0:c0 + bw], in_=gx[0:m, 0:bw],
                                     func=ACTF.Sign)
                nc.scalar.activation(out=sgy[0:m, c0:c0 + bw], in_=gy[0:m, 0:bw],
                                     func=ACTF.Sign)

            nc.gpsimd.tensor_add(msq[0:m, 1:gw + 1], sqx[0:m, 1:gw + 1],
                                 sqy[0:m, 1:gw + 1])
            # wrap columns (jnp.roll semantics on the W axis)
            nc.vector.tensor_copy(msq[0:m, 0:1], msq[0:m, gw:gw + 1])
            nc.vector.tensor_copy(msq[0:m, mw - 1:mw], msq[0:m, 1:2])

            m1 = maskp.tile([128, mw], U8, tag="m1", name=f"m1_{job.name}")
            is90 = maskp.tile([128, mw], U8, tag="is90", name=f"i9_{job.name}")
            sm = maskp.tile([128, mw], U8, tag="sm", name=f"sm_{job.name}")
            nc.vector.scalar_tensor_tensor(
                out=m1[0:m, 1:gw + 1], in0=sqx[0:m, 1:gw + 1], scalar=float(T2),
                in1=sqy[0:m, 1:gw + 1], op0=ALU.mult, op1=ALU.is_le)
            nc.vector.scalar_tensor_tensor(
                out=is90[0:m, 1:gw + 1], in0=sqy[0:m, 1:gw + 1], scalar=float(T2),
                in1=sqx[0:m, 1:gw + 1], op0=ALU.mult, op1=ALU.is_ge)
            nc.vector.tensor_tensor(
                out=sm[0:m, 1:gw + 1], in0=sgx[0:m, 1:gw + 1],
                in1=sgy[0:m, 1:gw + 1], op=ALU.is_equal)

            acc = accp.tile([128, 1], F32, tag="acc", name=f"acc_{job.name}")
            if m < 126:
                nc.gpsimd.memset(acc[:, :], 0.0)
            nc.vector.reduce_max(acc[0:m, 0:1], msq[0:m, 1:gw + 1],
                                 axis=mybir.AxisListType.X)
            return SimpleNamespace(msq=msq, m1=m1, is90=is90, sm=sm, acc=acc)

        def emit_pass_b(job, t, th_b, tl_b, t3_m):
            m = job.m
            msq, m1, is90, sm = t.msq, t.m1, t.is90, t.sm
            ut = udp.tile([128, mw], F32, tag="ut", name=f"ut_{job.name}")
            dt = udp.tile([128, mw], F32, tag="dt", name=f"dt_{job.name}")
            nc.gpsimd.memset(ut[0:1, :], 0.0)
            nc.sync.dma_start(dt[m - 1:m, :], msq[0:1, :])  # junk row
            nc.sync.dma_start(ut[1:m, :], msq[0:m - 1, :])
            nc.sync.dma_start(dt[0:m - 1, :], msq[1:m, :])

            nh = nmaxp.tile([128, mw], F32, tag="nh", name=f"nh_{job.name}")
            nv = nmaxp.tile([128, mw], F32, tag="nv", name=f"nv_{job.name}")
            n45 = nmaxp.tile([128, mw], F32, tag="n45", name=f"n45_{job.name}")
            n135 = nmaxp.tile([128, mw], F32, tag="n135", name=f"n135_{job.name}")
            nc.vector.tensor_tensor(out=nh[0:m, 1:gw + 1], in0=msq[0:m, 0:gw],
                                    in1=msq[0:m, 2:gw + 2], op=ALU.max)
            nc.vector.tensor_tensor(out=nv[0:m, 1:gw + 1], in0=ut[0:m, 1:gw + 1],
                                    in1=dt[0:m, 1:gw + 1], op=ALU.max)
            # 45 deg: NW & SE ; 135 deg: NE & SW
            nc.vector.tensor_tensor(out=n45[0:m, 1:gw + 1], in0=ut[0:m, 0:gw],
                                    in1=dt[0:m, 2:gw + 2], op=ALU.max)
            nc.vector.tensor_tensor(out=n135[0:m, 1:gw + 1], in0=ut[0:m, 2:gw + 2],
                                    in1=dt[0:m, 0:gw], op=ALU.max)

            nsel = selp.tile([128, mw], F32, tag="nsel", name=f"ns_{job.name}")
            ndg = selp.tile([128, mw], F32, tag="ndg", name=f"nd_{job.name}")
            nc.scalar.copy(ndg[0:m, 1:gw + 1], n135[0:m, 1:gw + 1])
            nc.vector.copy_predicated(ndg[0:m, 1:gw + 1], sm[0:m, 1:gw + 1],
                                      n45[0:m, 1:gw + 1])
            nc.scalar.copy(nsel[0:m, 1:gw + 1], nh[0:m, 1:gw + 1])
            nc.vector.copy_predicated(nsel[0:m, 1:gw + 1], m1[0:m, 1:gw + 1],
                                      ndg[0:m, 1:gw + 1])
            nc.vector.copy_predicated(nsel[0:m, 1:gw + 1], is90[0:m, 1:gw + 1],
                                      nv[0:m, 1:gw + 1])

            nth = selp.tile([128, mw], F32, tag="nth", name=f"nt_{job.name}")
            ntl = selp.tile([128, mw], F32, tag="ntl", name=f"ntl_{job.name}")
            nc.gpsimd.tensor_scalar(out=nth[0:m, 1:gw + 1], in0=nsel[0:m, 1:gw + 1],
                                    scalar1=th_b[0:m, 0:1], scalar2=None, op0=ALU.max)
            nc.gpsimd.tensor_scalar(out=ntl[0:m, 1:gw + 1], in0=nsel[0:m, 1:gw + 1],
                                    scalar1=tl_b[0:m, 0:1], scalar2=None, op0=ALU.max)
            strong = bfp.tile([128, mw], BF16, tag="strong", name=f"st_{job.name}")
            kb = bfp.tile([128, mw], BF16, tag="kb", name=f"kb_{job.name}")
            # box-sum matmul streams all 128 partitions / cols of strong
            nc.gpsimd.memset(strong[:, :], 0.0)
            nc.vector.tensor_tensor(out=strong[0:m, 1:gw + 1], in0=msq[0:m, 1:gw + 1],
                                    in1=nth[0:m, 1:gw + 1], op=ALU.is_ge)
            nc.vector.tensor_tensor(out=kb[0:m, 1:gw + 1], in0=msq[0:m, 1:gw + 1],
                                    in1=ntl[0:m, 1:gw + 1], op=ALU.is_ge)
            weak = bfp.tile([128, mw], BF16, tag="weak", name=f"wk_{job.name}")
            nc.gpsimd.tensor_tensor(out=weak[0:m, 1:gw + 1], in0=kb[0:m, 1:gw + 1],
                                    in1=strong[0:m, 1:gw + 1], op=ALU.subtract)

            pm = bfp.tile([128, mw], BF16, tag="pm", name=f"pm_{job.name}")
            for (c0, bw) in blocks:
                bx = ps.tile([128, 258], F32, tag="bx", name=f"bx_{job.name}_{c0}")
                nc.tensor.matmul(bx[0:m, 0:bw], t3_m[:, 0:m],
                                 strong[:, c0 - 1:c0 - 1 + bw], start=True, stop=False)
                nc.tensor.matmul(bx[0:m, 0:bw], t3_m[:, 0:m],
                                 strong[:, c0:c0 + bw], start=False, stop=False)
                nc.tensor.matmul(bx[0:m, 0:bw], t3_m[:, 0:m],
                                 strong[:, c0 + 1:c0 + 1 + bw], start=False, stop=True)
                nc.vector.tensor_scalar(out=pm[0:m, c0:c0 + bw], in0=bx[0:m, 0:bw],
                                        scalar1=0.5, scalar2=None, op0=ALU.is_ge)
            t2m = bfp.tile([128, mw], BF16, tag="t2m", name=f"t2_{job.name}")
            nc.gpsimd.tensor_mul(t2m[0:m, 2:gw], pm[0:m, 2:gw], weak[0:m, 2:gw])
            edg = bfp.tile([128, mw], BF16, tag="edg", name=f"ed_{job.name}")
            nc.gpsimd.tensor_add(edg[0:m, 2:gw], strong[0:m, 2:gw],
                                 t2m[0:m, 2:gw])
            ef = outp.tile([128, mw], F32, tag="ef", name=f"ef_{job.name}")
            nc.scalar.copy(ef[0:m, 2:gw], edg[0:m, 2:gw])

            for (pstart, sl, r0, nx) in job.segs:
                e0 = r0 + 3
                e1 = min(h, r0 + 2 + EDGE_ROWS)
                nc.sync.dma_start(o_d[sl, e0 - 1:e1, 0:w],
                                  ef[pstart + 2:pstart + 2 + (e1 - e0 + 1), 2:gw])

        reg_mats = (mat_s["vsn"], mat_s["vs"], mat_s["vd"], mat_s["vd2"])
        for g0 in range(0, dsh, gsz):
            group = list(range(g0, g0 + gsz))
            tiles = {}
            for sl in group:
                for si, (r0, m) in enumerate(full_strips):
                    job = SimpleNamespace(
                        name=f"{sl}_{si}", m=m, mats=reg_mats,
                        segs=[(0, sl, r0, 128)])
                    tiles[(sl, si)] = (job, emit_pass_a(job))
            if pack:
                (r0t, mt) = strips[-1]
                tjob = SimpleNamespace(
                    name=f"t{g0}", m=128,
                    mats=(mat_s["vsnp"], mat_s["vsp"], mat_s["vdp"], mat_s["vd2p"]),
                    segs=[(GSTRIDE * i, sl, r0t, mt + 2) for i, sl in enumerate(group)])
                ttiles = emit_pass_a(tjob)

            # thresholds + wrap patches per slice
            th_of, tl_of = {}, {}
            for i, sl in enumerate(group):
                # per-partition max tree across this slice's strips, then one
                # cross-partition reduce
                per = [tiles[(sl, si)][1].acc for si in range(len(full_strips))]
                mmin = min(tiles[(sl, si)][0].m for si in range(len(full_strips)))                     if full_strips else 0
                if pack:
                    (r0t, mt) = strips[-1]
                    ax = accp.tile([128, 1], F32, tag="acct", name=f"at_{sl}")
                    nc.gpsimd.memset(ax[:, :], 0.0)
                    nc.sync.dma_start(ax[0:mt, 0:1],
                                      ttiles.acc[GSTRIDE * i:GSTRIDE * i + mt, 0:1])
                    per.append(ax)
                while len(per) > 1:
                    nxt = []
                    for j in range(0, len(per) - 1, 2):
                        r = accp.tile([128, 1], F32, tag="mxc",
                                      name=f"mx_{sl}_{id(per[j])}")
                        nc.vector.tensor_tensor(out=r[0:126, 0:1],
                                                in0=per[j][0:126, 0:1],
                                                in1=per[j + 1][0:126, 0:1],
                                                op=ALU.max)
                        nxt.append(r)
                    if len(per) % 2:
                        nxt.append(per[-1])
                    per = nxt
                mx = accp.tile([128, 1], F32, tag="accg", name=f"agf_{sl}")
                nc.gpsimd.partition_all_reduce(mx[0:126, 0:1], per[0][0:126, 0:1],
                                               channels=126,
                                               reduce_op=bass_isa.ReduceOp.max)
                th1 = thrp.tile([1, 1], F32, tag="th1", name=f"th1_{sl}")
                tl1 = thrp.tile([1, 1], F32, tag="tl1", name=f"tl1_{sl}")
                nc.vector.tensor_scalar(out=th1[0:1, 0:1], in0=mx[0:1, 0:1],
                                        scalar1=float(CSQ), scalar2=None, op0=ALU.mult)
                nc.vector.tensor_scalar(out=tl1[0:1, 0:1], in0=th1[0:1, 0:1],
                                        scalar1=float(DSQ), scalar2=None, op0=ALU.mult)
                th_b = thrp.tile([128, 1], F32, tag="thb", name=f"thb_{sl}")
                tl_b = thrp.tile([128, 1], F32, tag="tlb", name=f"tlb_{sl}")
                nc.gpsimd.partition_broadcast(th_b, th1[0:1, 0:1])
                nc.gpsimd.partition_broadcast(tl_b, tl1[0:1, 0:1])
                th_of[sl], tl_of[sl] = (th1, th_b), (tl1, tl_b)

                # wrap rows: first strip's row -1 <- grid row h+1;
                # last strip's row h+2 <- grid row 0
                first_msq = tiles[(sl, 0)][1].msq
                if pack:
                    (r0t, mt) = strips[-1]
                    last_msq = ttiles.msq
                    p_base = GSTRIDE * i
                else:
                    last_msq = tiles[(sl, len(full_strips) - 1)][1].msq
                    p_base = 0
                r0l = strips[-1][0]
                p_last = p_base + (h + 1) - (r0l + 1)
                p_zero = 0 - (strips[0][0] + 1)
                nc.sync.dma_start(first_msq[0:1, :], last_msq[p_last:p_last + 1, :])
                nc.sync.dma_start(last_msq[p_last + 1:p_last + 2, :],
                                  first_msq[p_zero:p_zero + 1, :])

            # pass B
            for sl in group:
                (_, th_b) = th_of[sl]
                (_, tl_b) = tl_of[sl]
                for si in range(len(full_strips)):
                    (job, t) = tiles[(sl, si)]
                    emit_pass_b(job, t, th_b, tl_b, mat_s["t3"])
            if pack:
                # packed per-partition thresholds
                thp = thrp.tile([128, 1], F32, tag="thp", name=f"thp_{g0}")
                tlp = thrp.tile([128, 1], F32, tag="tlp", name=f"tlp_{g0}")
                nc.gpsimd.memset(thp[:, :], 0.0)
                nc.gpsimd.memset(tlp[:, :], 0.0)
                (r0t, mt) = strips[-1]
                for i, sl in enumerate(group):
                    (_, th_bs) = th_of[sl]
                    (_, tl_bs) = tl_of[sl]
                    nc.sync.dma_start(thp[GSTRIDE * i:GSTRIDE * i + mt, 0:1],
                                      th_bs[0:mt, 0:1])
                    nc.sync.dma_start(tlp[GSTRIDE * i:GSTRIDE * i + mt, 0:1],
                                      tl_bs[0:mt, 0:1])
                emit_pass_b(tjob, ttiles, thp, tlp, mat_s["t3p"])

    nc.compile()
    return nc


_NC_CACHE = {}


def _get_nc(dsh, h, w):
    key = (dsh, h, w)
    if key not in _NC_CACHE:
        _NC_CACHE[key] = build_nc(dsh, h, w)
    return _NC_CACHE[key]


def kernel(x, gk=None, sobel_x=None, sobel_y=None):
    """Full-input entry point: x [128, 512, 512] f32 -> edges [128, 512, 512] f32."""
    x = np.ascontiguousarray(np.asarray(x), dtype=np.float32)
    nc = _get_nc(D_SH, x.shape[1], x.shape[2])
    mats = _stencil_mats(x.shape[1])
    in_maps = []
    for c in range(N_CORES):
        m = {"x": x[c * D_SH:(c + 1) * D_SH]}
        m.update(mats)
        in_maps.append(m)
    res = bass_utils.run_bass_kernel_spmd(nc, in_maps, core_ids=list(range(N_CORES)))
    out = np.concatenate([res.results[c]["o"] for c in range(N_CORES)], axis=0)
    return out.astype(np.float32)


# revision 31
# speedup vs baseline: 1.9161x; 1.9161x over previous
"""Canny edge filter (nms_detection) Trainium2 Bass kernel.

Full inputs: x [128, 512, 512] f32 (plus 1x1 gaussian + sobel kernels, which
are compile-time constants here). Output: [128, 512, 512] f32 binary edges.

Strategy: shard the 128 slices across 8 cores (16 per core). Each slice is
independent (3x3 stencils + per-slice max). All math is done in the
squared-magnitude domain (no sqrt / arctan2 needed):
  - gx, gy via fp32 TensorE matmuls with banded stencil matrices
    (vertical part) and column-shifted access patterns (horizontal part).
  - sqx, sqy via ScalarE Square (exact), msq = sqx + sqy.
  - NMS direction via comparisons: t^2*sqx <= sqy etc. (t = tan 22.5deg).
  - neighbor access via DMA partition-shifted copies of msq + col offsets.
  - per-slice max of msq == per-slice max of NMS'd mag^2 (the argmax always
    survives NMS), so thresholds are computed in pass A.
  - hysteresis: 3x3 box-sum of strong on PE in bf16 (exact for 0/1 data).
  - the short tail strips of 4 slices are packed into one 128-partition tile
    (32-partition groups, block-diagonal stencil matrices).
"""
import sys
import math
from contextlib import ExitStack
from types import SimpleNamespace

sys.path.insert(0, "/opt/trn_rl_repo")

import numpy as np
import ml_dtypes

import concourse.bass as bass
import concourse.bacc as bacc
import concourse.bass_isa as bass_isa
import concourse.mybir as mybir
import concourse.tile as tile
from concourse import bass_utils

F32 = mybir.dt.float32
BF16 = mybir.dt.bfloat16
U8 = mybir.dt.uint8
ALU = mybir.AluOpType
ACTF = mybir.ActivationFunctionType

D, H, W = 128, 512, 512
N_CORES = 8
D_SH = D // N_CORES

# constants matching the reference's f32 arithmetic boundaries
T2 = np.float32((math.sqrt(2.0) - 1.0) ** 2)          # tan^2(22.5 deg)
CSQ = np.float32(np.float64(np.float32(0.05)) ** 2)    # HIGH_T^2
DSQ = np.float32(np.float64(np.float32(0.01)) ** 2)    # LOW_T^2

EDGE_ROWS = 122  # edge rows produced per full strip (128 partitions - 6 halo)
GP_OFFLOAD = False  # route some elementwise ops to GpSimd (HW-measured slower)
GSTRIDE = 32     # partition stride of packed tail groups


def _band_mats(specs, ncols):
    """Build banded lhsT matrices [128, ncols] from (col, row, weight) spec fn.

    specs: list of (qstart, qcount) bands; each band places, for local q in
    [0, qcount): weights at rows qstart+q+dj for dj, w in the stencil."""
    vs = np.zeros((128, ncols), np.float32)
    vd = np.zeros((128, ncols), np.float32)
    t3 = np.zeros((128, ncols), np.float32)
    for (qs, qc) in specs:
        for j in range(qc):
            q = qs + j
            vs[q, q] = 1.0
            vs[q + 1, q] = 2.0
            vs[q + 2, q] = 1.0
            vd[q, q] = -1.0
            vd[q + 2, q] = 1.0
            # centered ones band, clipped to this group's rows
            if j > 0:
                t3[q - 1, q] = 1.0
            t3[q, q] = 1.0
            if j < qc - 1 or True:
                t3[q + 1, q] = 1.0
    return vs, -vs, vd, 2.0 * vd, t3.astype(ml_dtypes.bfloat16)


def _stencil_mats(h=H):
    """Regular + packed-tail matrices for height h."""
    strips = _strips(h)
    reg = _band_mats([(0, 126)], 128)
    mats = {"vs": reg[0], "vsn": reg[1], "vd": reg[2], "vd2": reg[3], "t3": reg[4]}
    if _pack_ok(h, 4):
        mt = strips[-1][1]
        pk = _band_mats([(g * GSTRIDE, mt) for g in range(4)], 128)
        mats.update({"vsp": pk[0], "vsnp": pk[1], "vdp": pk[2],
                     "vd2p": pk[3], "t3p": pk[4]})
    return mats


def _strips(h):
    """Strip table: (R0, M). R0 = grid row of x partition 0.
    msq rows R0+1 .. R0+M, edge rows R0+3 .. min(h, R0+2+EDGE_ROWS)."""
    n = max(1, math.ceil(h / EDGE_ROWS))
    out = []
    for s in range(n):
        r0 = EDGE_ROWS * s - 2
        m = min(126, (h + 2) - (EDGE_ROWS * s - 1) + 1)
        out.append((r0, m))
    return out


def _pack_ok(h, gsz):
    strips = _strips(h)
    return len(strips) >= 2 and strips[-1][1] + 2 <= GSTRIDE - 1


def build_nc(dsh, h, w, reps=1):
    """Build the per-core Bass program: x [dsh, h, w] -> out [dsh, h, w]."""
    gw = w + 2          # grid width
    xw = w + 6          # x tile width; x col t <-> grid col t-2
    mw = w + 4          # msq tile width; msq col t <-> grid col t-1
    blocks = []
    c = 1
    while c < gw + 1:
        bw = min(258, gw + 1 - c)
        blocks.append((c, bw))
        c += bw
    strips = _strips(h)
    pack = _pack_ok(h, 4) and dsh % 4 == 0
    gsz = 4 if pack else 1
    full_strips = strips[:-1] if pack else strips
    mats_np = _stencil_mats(h)

    nc = bacc.Bacc(trn_type="TRN2")
    x_d = nc.dram_tensor("x", [dsh, h, w], F32, kind="ExternalInput")
    mat_d = {}
    for name, arr in mats_np.items():
        dt = BF16 if arr.dtype == ml_dtypes.bfloat16 else F32
        mat_d[name] = nc.dram_tensor(name, list(arr.shape), dt, kind="ExternalInput")
    o_d = nc.dram_tensor("o", [dsh, h, w], F32, kind="ExternalOutput")

    with ExitStack() as ctx:
        tc = ctx.enter_context(tile.TileContext(nc))
        consts = ctx.enter_context(tc.tile_pool(name="consts", bufs=1))
        xp = ctx.enter_context(tc.tile_pool(name="xp", bufs=3))
        ps = ctx.enter_context(tc.tile_pool(name="ps", bufs=2, space="PSUM"))
        sqp = ctx.enter_context(tc.tile_pool(name="sqp", bufs=2))
        msqp = ctx.enter_context(tc.tile_pool(name="msqp", bufs=20))
        maskp = ctx.enter_context(tc.tile_pool(name="maskp", bufs=20))
        accp = ctx.enter_context(tc.tile_pool(name="accp", bufs=24))
        thrp = ctx.enter_context(tc.tile_pool(name="thrp", bufs=6))
        udp = ctx.enter_context(tc.tile_pool(name="udp", bufs=2))
        nmaxp = ctx.enter_context(tc.tile_pool(name="nmaxp", bufs=2))
        selp = ctx.enter_context(tc.tile_pool(name="selp", bufs=2))
        bfp = ctx.enter_context(tc.tile_pool(name="bfp", bufs=2))
        outp = ctx.enter_context(tc.tile_pool(name="outp", bufs=3))

        mat_s = {}
        for name, dram in mat_d.items():
            t = consts.tile(list(mats_np[name].shape),
                            BF16 if name.startswith("t3") else F32,
                            tag=name, name=f"c_{name}")
            nc.sync.dma_start(t, dram[:])
            mat_s[name] = t

        loop_ctx = tc.For_i(0, reps, 1) if reps > 1 else None
        if loop_ctx is not None:
            ctx.enter_context(loop_ctx)

        def emit_pass_a(job):
            """Pass A for one tile-job. Returns tiles namespace."""
            m = job.m
            xt = xp.tile([128, xw], F32, tag="x", name=f"x_{job.name}")
            # full clear unless a single segment's image rows + top-pad memset
            # cover all 128 partitions
            clear_x = not (len(job.segs) == 1 and job.segs[0][0] == 0
                           and min(h, job.segs[0][2] + 127) - job.segs[0][2] == 127)
            if clear_x:
                nc.gpsimd.memset(xt[:, :], 0.0)
            else:
                nc.gpsimd.memset(xt[:, 0:3], 0.0)
                nc.gpsimd.memset(xt[:, xw - 3:xw], 0.0)
                for (pstart, sl, r0, nx) in job.segs:
                    img_lo = max(1, r0) - r0
                    if img_lo > 0:
                        assert pstart == 0
                        nc.gpsimd.memset(xt[0:img_lo, :], 0.0)
            for (pstart, sl, r0, nx) in job.segs:
                img_lo = max(1, r0) - r0
                img_hi = min(h, r0 + nx - 1) - r0
                nc.sync.dma_start(
                    xt[pstart + img_lo:pstart + img_hi + 1, 3:w + 3],
                    x_d[sl, r0 + img_lo - 1:r0 + img_hi, 0:w])

            msq = msqp.tile([128, mw], F32, tag="msq", name=f"msq_{job.name}")
            sqx = sqp.tile([128, mw], F32, tag="sqx", name=f"sqx_{job.name}")
            sqy = sqp.tile([128, mw], F32, tag="sqy", name=f"sqy_{job.name}")
            sgx = sqp.tile([128, mw], BF16, tag="sgx", name=f"sgx_{job.name}")
            sgy = sqp.tile([128, mw], BF16, tag="sgy", name=f"sgy_{job.name}")
            vsn_m, vs_m, vd_m, vd2_m = job.mats
            for (c0, bw) in blocks:
                xl, xc, xr = c0, c0 + 1, c0 + 2
                gx = ps.tile([128, 258], F32, tag="gx", name=f"gx_{job.name}_{c0}")
                gy = ps.tile([128, 258], F32, tag="gy", name=f"gy_{job.name}_{c0}")
                nc.tensor.matmul(gx[0:m, 0:bw], vsn_m[:, 0:m], xt[:, xl:xl + bw],
                                 start=True, stop=False)
                nc.tensor.matmul(gx[0:m, 0:bw], vs_m[:, 0:m], xt[:, xr:xr + bw],
                                 start=False, stop=True)
                nc.tensor.matmul(gy[0:m, 0:bw], vd_m[:, 0:m], xt[:, xl:xl + bw],
                                 start=True, stop=False)
                nc.tensor.matmul(gy[0:m, 0:bw], vd2_m[:, 0:m], xt[:, xc:xc + bw],
                                 start=False, stop=False)
                nc.tensor.matmul(gy[0:m, 0:bw], vd_m[:, 0:m], xt[:, xr:xr + bw],
                                 start=False, stop=True)
                nc.scalar.activation(out=sqx[0:m, c0:c0 + bw], in_=gx[0:m, 0:bw],
                                     func=ACTF.Square)
                nc.scalar.activation(out=sqy[0:m, c0:c0 + bw], in_=gy[0:m, 0:bw],
                                     func=ACTF.Square)
                nc.scalar.activation(out=sgx[0:m, c0:c0 + bw], in_=gx[0:m, 0:bw],
                                     func=ACTF.Sign)
                nc.scalar.activation(out=sgy[0:m, c0:c0 + bw], in_=gy[0:m, 0:bw],
                                     func=ACTF.Sign)

            eng = nc.gpsimd if GP_OFFLOAD else nc.vector
            eng.tensor_add(msq[0:m, 1:gw + 1], sqx[0:m, 1:gw + 1],
                           sqy[0:m, 1:gw + 1])
            # wrap columns (jnp.roll semantics on the W axis)
            nc.vector.tensor_copy(msq[0:m, 0:1], msq[0:m, gw:gw + 1])
            nc.vector.tensor_copy(msq[0:m, mw - 1:mw], msq[0:m, 1:2])

            m1 = maskp.tile([128, mw], U8, tag="m1", name=f"m1_{job.name}")
            is90 = maskp.tile([128, mw], U8, tag="is90", name=f"i9_{job.name}")
            sm = maskp.tile([128, mw], U8, tag="sm", name=f"sm_{job.name}")
            nc.vector.scalar_tensor_tensor(
                out=m1[0:m, 1:gw + 1], in0=sqx[0:m, 1:gw + 1], scalar=float(T2),
                in1=sqy[0:m, 1:gw + 1], op0=ALU.mult, op1=ALU.is_le)
            nc.vector.scalar_tensor_tensor(
                out=is90[0:m, 1:gw + 1], in0=sqy[0:m, 1:gw + 1], scalar=float(T2),
                in1=sqx[0:m, 1:gw + 1], op0=ALU.mult, op1=ALU.is_ge)
            nc.vector.tensor_tensor(
                out=sm[0:m, 1:gw + 1], in0=sgx[0:m, 1:gw + 1],
                in1=sgy[0:m, 1:gw + 1], op=ALU.is_equal)

            acc = accp.tile([128, 1], F32, tag="acc", name=f"acc_{job.name}")
            if m < 126:
                nc.gpsimd.memset(acc[:, :], 0.0)
            nc.vector.reduce_max(acc[0:m, 0:1], msq[0:m, 1:gw + 1],
                                 axis=mybir.AxisListType.X)
            return SimpleNamespace(msq=msq, m1=m1, is90=is90, sm=sm, acc=acc)

        def emit_pass_b(job, t, th_b, tl_b, t3_m):
            m = job.m
            msq, m1, is90, sm = t.msq, t.m1, t.is90, t.sm
            ut = udp.tile([128, mw], F32, tag="ut", name=f"ut_{job.name}")
            dt = udp.tile([128, mw], F32, tag="dt", name=f"dt_{job.name}")
            nc.sync.dma_start(ut[0:1, :], msq[0:1, :])    # junk row, never consumed
            nc.sync.dma_start(dt[m - 1:m, :], msq[0:1, :])  # junk row
            nc.sync.dma_start(ut[1:m, :], msq[0:m - 1, :])
            nc.sync.dma_start(dt[0:m - 1, :], msq[1:m, :])

            nh = nmaxp.tile([128, mw], F32, tag="nh", name=f"nh_{job.name}")
            nv = nmaxp.tile([128, mw], F32, tag="nv", name=f"nv_{job.name}")
            n45 = nmaxp.tile([128, mw], F32, tag="n45", name=f"n45_{job.name}")
            n135 = nmaxp.tile([128, mw], F32, tag="n135", name=f"n135_{job.name}")
            nc.vector.tensor_tensor(out=nh[0:m, 1:gw + 1], in0=msq[0:m, 0:gw],
                                    in1=msq[0:m, 2:gw + 2], op=ALU.max)
            nc.vector.tensor_tensor(out=nv[0:m, 1:gw + 1], in0=ut[0:m, 1:gw + 1],
                                    in1=dt[0:m, 1:gw + 1], op=ALU.max)
            # 45 deg: NW & SE ; 135 deg: NE & SW
            nc.vector.tensor_tensor(out=n45[0:m, 1:gw + 1], in0=ut[0:m, 0:gw],
                                    in1=dt[0:m, 2:gw + 2], op=ALU.max)
            nc.vector.tensor_tensor(out=n135[0:m, 1:gw + 1], in0=ut[0:m, 2:gw + 2],
                                    in1=dt[0:m, 0:gw], op=ALU.max)

            nsel = selp.tile([128, mw], F32, tag="nsel", name=f"ns_{job.name}")
            ndg = selp.tile([128, mw], F32, tag="ndg", name=f"nd_{job.name}")
            nc.scalar.copy(ndg[0:m, 1:gw + 1], n135[0:m, 1:gw + 1])
            nc.vector.copy_predicated(ndg[0:m, 1:gw + 1], sm[0:m, 1:gw + 1],
                                      n45[0:m, 1:gw + 1])
            nc.scalar.copy(nsel[0:m, 1:gw + 1], nh[0:m, 1:gw + 1])
            nc.vector.copy_predicated(nsel[0:m, 1:gw + 1], m1[0:m, 1:gw + 1],
                                      ndg[0:m, 1:gw + 1])
            nc.vector.copy_predicated(nsel[0:m, 1:gw + 1], is90[0:m, 1:gw + 1],
                                      nv[0:m, 1:gw + 1])

            nth = selp.tile([128, mw], F32, tag="nth", name=f"nt_{job.name}")
            ntl = selp.tile([128, mw], F32, tag="ntl", name=f"ntl_{job.name}")
            eng = nc.gpsimd if GP_OFFLOAD else nc.vector
            eng.tensor_scalar(out=nth[0:m, 1:gw + 1], in0=nsel[0:m, 1:gw + 1],
                              scalar1=th_b[0:m, 0:1], scalar2=None, op0=ALU.max)
            eng.tensor_scalar(out=ntl[0:m, 1:gw + 1], in0=nsel[0:m, 1:gw + 1],
                              scalar1=tl_b[0:m, 0:1], scalar2=None, op0=ALU.max)
            strong = bfp.tile([128, mw], BF16, tag="strong", name=f"st_{job.name}")
            kb = bfp.tile([128, mw], BF16, tag="kb", name=f"kb_{job.name}")
            if m < 128:
                # box-sum matmul streams all 128 partitions / cols of strong
                nc.gpsimd.memset(strong[:, :], 0.0)
            else:
                nc.gpsimd.memset(strong[:, 0:1], 0.0)
                nc.gpsimd.memset(strong[:, mw - 1:mw], 0.0)
            nc.vector.tensor_tensor(out=strong[0:m, 1:gw + 1], in0=msq[0:m, 1:gw + 1],
                                    in1=nth[0:m, 1:gw + 1], op=ALU.is_ge)
            nc.vector.tensor_tensor(out=kb[0:m, 1:gw + 1], in0=msq[0:m, 1:gw + 1],
                                    in1=ntl[0:m, 1:gw + 1], op=ALU.is_ge)
            weak = bfp.tile([128, mw], BF16, tag="weak", name=f"wk_{job.name}")
            eng.tensor_tensor(out=weak[0:m, 1:gw + 1], in0=kb[0:m, 1:gw + 1],
                              in1=strong[0:m, 1:gw + 1], op=ALU.subtract)

            pm = bfp.tile([128, mw], BF16, tag="pm", name=f"pm_{job.name}")
            for (c0, bw) in blocks:
                bx = ps.tile([128, 258], F32, tag="bx", name=f"bx_{job.name}_{c0}")
                nc.tensor.matmul(bx[0:m, 0:bw], t3_m[:, 0:m],
                                 strong[:, c0 - 1:c0 - 1 + bw], start=True, stop=False)
                nc.tensor.matmul(bx[0:m, 0:bw], t3_m[:, 0:m],
                                 strong[:, c0:c0 + bw], start=False, stop=False)
                nc.tensor.matmul(bx[0:m, 0:bw], t3_m[:, 0:m],
                                 strong[:, c0 + 1:c0 + 1 + bw], start=False, stop=True)
                nc.vector.tensor_scalar(out=pm[0:m, c0:c0 + bw], in0=bx[0:m, 0:bw],
                                        scalar1=0.5, scalar2=None, op0=ALU.is_ge)
            t2m = bfp.tile([128, mw], BF16, tag="t2m", name=f"t2_{job.name}")
            eng.tensor_mul(t2m[0:m, 2:gw], pm[0:m, 2:gw], weak[0:m, 2:gw])
            edg = bfp.tile([128, mw], BF16, tag="edg", name=f"ed_{job.name}")
            eng.tensor_add(edg[0:m, 2:gw], strong[0:m, 2:gw],
                           t2m[0:m, 2:gw])
            ef = outp.tile([128, mw], F32, tag="ef", name=f"ef_{job.name}")
            nc.scalar.copy(ef[0:m, 2:gw], edg[0:m, 2:gw])

            for (pstart, sl, r0, nx) in job.segs:
                e0 = r0 + 3
                e1 = min(h, r0 + 2 + EDGE_ROWS)
                nc.sync.dma_start(o_d[sl, e0 - 1:e1, 0:w],
                                  ef[pstart + 2:pstart + 2 + (e1 - e0 + 1), 2:gw])

        reg_mats = (mat_s["vsn"], mat_s["vs"], mat_s["vd"], mat_s["vd2"])
        for g0 in range(0, dsh, gsz):
            group = list(range(g0, g0 + gsz))
            tiles = {}
            for sl in group:
                for si, (r0, m) in enumerate(full_strips):
                    job = SimpleNamespace(
                        name=f"{sl}_{si}", m=128 if m == 126 else m, mats=reg_mats,
                        segs=[(0, sl, r0, 128)])
                    tiles[(sl, si)] = (job, emit_pass_a(job))
            if pack:
                (r0t, mt) = strips[-1]
                tjob = SimpleNamespace(
                    name=f"t{g0}", m=128,
                    mats=(mat_s["vsnp"], mat_s["vsp"], mat_s["vdp"], mat_s["vd2p"]),
                    segs=[(GSTRIDE * i, sl, r0t, mt + 2) for i, sl in enumerate(group)])
                ttiles = emit_pass_a(tjob)

            # thresholds + wrap patches per slice
            th_of, tl_of = {}, {}
            for i, sl in enumerate(group):
                # per-partition max tree across this slice's strips, then one
                # cross-partition reduce
                per = [tiles[(sl, si)][1].acc for si in range(len(full_strips))]
                mmin = min(tiles[(sl, si)][0].m for si in range(len(full_strips)))                     if full_strips else 0
                if pack:
                    (r0t, mt) = strips[-1]
                    ax = accp.tile([128, 1], F32, tag="acct", name=f"at_{sl}")
                    nc.gpsimd.memset(ax[:, :], 0.0)
                    nc.sync.dma_start(ax[0:mt, 0:1],
                                      ttiles.acc[GSTRIDE * i:GSTRIDE * i + mt, 0:1])
                    per.append(ax)
                while len(per) > 1:
                    nxt = []
                    for j in range(0, len(per) - 1, 2):
                        r = accp.tile([128, 1], F32, tag="mxc",
                                      name=f"mx_{sl}_{id(per[j])}")
                        nc.vector.tensor_tensor(out=r[0:126, 0:1],
                                                in0=per[j][0:126, 0:1],
                                                in1=per[j + 1][0:126, 0:1],
                                                op=ALU.max)
                        nxt.append(r)
                    if len(per) % 2:
                        nxt.append(per[-1])
                    per = nxt
                mx = accp.tile([128, 1], F32, tag="accg", name=f"agf_{sl}")
                nc.gpsimd.partition_all_reduce(mx[0:126, 0:1], per[0][0:126, 0:1],
                                               channels=126,
                                               reduce_op=bass_isa.ReduceOp.max)
                th1 = thrp.tile([1, 1], F32, tag="th1", name=f"th1_{sl}")
                tl1 = thrp.tile([1, 1], F32, tag="tl1", name=f"tl1_{sl}")
                nc.vector.tensor_scalar(out=th1[0:1, 0:1], in0=mx[0:1, 0:1],
                                        scalar1=float(CSQ), scalar2=None, op0=ALU.mult)
                nc.vector.tensor_scalar(out=tl1[0:1, 0:1], in0=th1[0:1, 0:1],
                                        scalar1=float(DSQ), scalar2=None, op0=ALU.mult)
                th_b = thrp.tile([128, 1], F32, tag="thb", name=f"thb_{sl}")
                tl_b = thrp.tile([128, 1], F32, tag="tlb", name=f"tlb_{sl}")
                nc.gpsimd.partition_broadcast(th_b, th1[0:1, 0:1])
                nc.gpsimd.partition_broadcast(tl_b, tl1[0:1, 0:1])
                th_of[sl], tl_of[sl] = (th1, th_b), (tl1, tl_b)

                # wrap rows: first strip's row -1 <- grid row h+1;
                # last strip's row h+2 <- grid row 0
                first_msq = tiles[(sl, 0)][1].msq
                if pack:
                    (r0t, mt) = strips[-1]
                    last_msq = ttiles.msq
                    p_base = GSTRIDE * i
                else:
                    last_msq = tiles[(sl, len(full_strips) - 1)][1].msq
                    p_base = 0
                r0l = strips[-1][0]
                p_last = p_base + (h + 1) - (r0l + 1)
                p_zero = 0 - (strips[0][0] + 1)
                nc.sync.dma_start(first_msq[0:1, :], last_msq[p_last:p_last + 1, :])
                nc.sync.dma_start(last_msq[p_last + 1:p_last + 2, :],
                                  first_msq[p_zero:p_zero + 1, :])

            # pass B
            for sl in group:
                (_, th_b) = th_of[sl]
                (_, tl_b) = tl_of[sl]
                for si in range(len(full_strips)):
                    (job, t) = tiles[(sl, si)]
                    emit_pass_b(job, t, th_b, tl_b, mat_s["t3"])
            if pack:
                # packed per-partition thresholds
                thp = thrp.tile([128, 1], F32, tag="thp", name=f"thp_{g0}")
                tlp = thrp.tile([128, 1], F32, tag="tlp", name=f"tlp_{g0}")
                nc.gpsimd.memset(thp[:, :], 0.0)
                nc.gpsimd.memset(tlp[:, :], 0.0)
                (r0t, mt) = strips[-1]
                for i, sl in enumerate(group):
                    (_, th_bs) = th_of[sl]
                    (_, tl_bs) = tl_of[sl]
                    nc.sync.dma_start(thp[GSTRIDE * i:GSTRIDE * i + mt, 0:1],
                                      th_bs[0:mt, 0:1])
                    nc.sync.dma_start(tlp[GSTRIDE * i:GSTRIDE * i + mt, 0:1],
                                      tl_bs[0:mt, 0:1])
                emit_pass_b(tjob, ttiles, thp, tlp, mat_s["t3p"])

    nc.compile()
    return nc


_NC_CACHE = {}


def _get_nc(dsh, h, w):
    key = (dsh, h, w)
    if key not in _NC_CACHE:
        _NC_CACHE[key] = build_nc(dsh, h, w)
    return _NC_CACHE[key]


def kernel(x, gk=None, sobel_x=None, sobel_y=None):
    """Full-input entry point: x [128, 512, 512] f32 -> edges [128, 512, 512] f32."""
    x = np.ascontiguousarray(np.asarray(x), dtype=np.float32)
    nc = _get_nc(D_SH, x.shape[1], x.shape[2])
    mats = _stencil_mats(x.shape[1])
    in_maps = []
    for c in range(N_CORES):
        m = {"x": x[c * D_SH:(c + 1) * D_SH]}
        m.update(mats)
        in_maps.append(m)
    res = bass_utils.run_bass_kernel_spmd(nc, in_maps, core_ids=list(range(N_CORES)))
    out = np.concatenate([res.results[c]["o"] for c in range(N_CORES)], axis=0)
    return out.astype(np.float32)
